# revision 1
# baseline (speedup 1.0000x reference)
"""LocSE (RandLA-Net local spatial encoding) Trainium2 Bass kernel.

Full-input contract: kernel(**inputs) takes the unsharded numpy inputs and
returns the full (B, N, K, 2F) float32 output. Internally the work is
data-parallel across 8 NeuronCores: core c handles sample b = c//2 and half
of the N points (h = c%2). Each core holds the full per-sample gather table
since neighbor indices span the whole sample.

The neighbor gather uses the GPSIMD SWDGE dma_gather, whose indices are
int16 (< 32768). N = 65536 exceeds that, so the table packs point PAIRS:
row m (512B stride) holds [feats[2m] | pc[2m] | feats[2m+1] | pc[2m+1]];
the device gathers row idx>>1 and selects the half with an exact
arithmetic parity select: out = lo + (hi - lo) * (idx & 1).

Per 512-point tile (4 sub-groups of 128 points):
  1. one dma_gather of 4096 pair-rows (one per (point, k)),
  2. parity select of feats into the output tile and of neighbor pc into X,
  3. dx/dy/norm on DVE + ACT(sqrt),
  4. the 7->32 MLP folded over relp = Kpc - n_points:
       r = relu(Kpc.(W02+W46) + np.(W24-W46) + ||relp||.W6 + b)
     i.e. ONE matmul per 128-point group against a host-built block-diagonal
     (32 x 256) weight, fed by the PE-transposed [cx, cy, (npx,npy,nrm)x8, 1]
     matrix,
  5. ACT relu lands r next to the selected feats; one contiguous 1MB DMA
     stores the finished (point, k, 2F) rows.
"""

import sys

if "/opt/trn_rl_repo" not in sys.path:
    sys.path.insert(0, "/opt/trn_rl_repo")

import numpy as np

B, N, K = 4, 65536, 8
DIMS, F = 2, 32
TROW = F + DIMS  # 34 f32: feats | pc
PAIR = 128  # pair-table row: 2*TROW data + pad, 512B stride
NCORES = 8
ROWS_PER_CORE = B * N // NCORES  # 32768
S = 4  # 128-point sub-groups per tile
SK = S * K
PTS_PER_TILE = 128 * S
NIDX = 128 * SK  # gathers per tile
XC = 3 * K + 3  # 27 data columns: cx, cy, (npx, npy, nrm) x K, one
XCP = 32  # padded so every matmul reads at base partition 0


def build_program(nrows, ntable, bufs=3, timing_variant=False, single_packet=True, loops=1, skip_gather=False, gather_only=False):
    """Build the per-core Bass program (same program on all cores)."""
    import concourse.bacc as bacc
    import concourse.mybir as mybir
    import concourse.tile as tile
    from concourse.masks import make_identity

    f32 = mybir.dt.float32
    i16 = mybir.dt.int16
    ntiles = nrows // PTS_PER_TILE
    assert nrows % PTS_PER_TILE == 0 and ntable % 2 == 0

    nc = bacc.Bacc(None)

    T2_d = nc.dram_tensor("T2", [ntable // 2, PAIR], f32, kind="ExternalInput")
    # One gather per 128-point sub-group: the SWDGE ring handles at most
    # 1024 descriptors per dma_gather (HW-validated; 1280 wedges the device).
    idxw_d = nc.dram_tensor(
        "idxw", [ntiles * S * 128, 128 * K // 16], i16, kind="ExternalInput"
    )
    parw_d = nc.dram_tensor("parw", [ntiles, 128, SK], f32, kind="ExternalInput")
    pcc_d = nc.dram_tensor("pcc", [nrows, DIMS], f32, kind="ExternalInput")
    wf_d = nc.dram_tensor("Wf", [XCP, K * F], f32, kind="ExternalInput")
    if timing_variant:
        out_d = nc.dram_tensor("out_int", [nrows, K, 2 * F], f32)
        dummy_d = nc.dram_tensor("tout", [128, 1], f32, kind="ExternalOutput")
    else:
        out_d = nc.dram_tensor("out", [nrows, K, 2 * F], f32, kind="ExternalOutput")

    idxw_r = idxw_d[:, :].rearrange("(t s p) m -> t p s m", s=S, p=128)
    pcc_r = pcc_d[:, :].rearrange("(t s p) d -> t p s d", s=S, p=128)
    out_r = out_d[:, :, :].rearrange("(t s p) k f -> t p s (k f)", s=S, p=128)

    with tile.TileContext(nc) as tc:
        with (
            tc.tile_pool(name="persist", bufs=1) as persist,
            tc.tile_pool(name="sbuf", bufs=bufs) as pool,
            tc.tile_pool(name="gbuf", bufs=2) as gpool,
            tc.tile_pool(name="psum", bufs=2, space="PSUM") as psum,
        ):
            wf_sb = persist.tile([XCP, K * F], f32)
            nc.sync.dma_start(wf_sb[:], wf_d[:, :])
            ident = persist.tile([128, 128], f32)
            make_identity(nc, ident[:])
            # Tail constants for X columns 26..31: [1, 0, 0, 0, 0, 0].
            onez = persist.tile([128, XCP - XC + 1], f32)
            nc.gpsimd.memset(onez[:], 0.0)
            nc.gpsimd.memset(onez[:, 0:1], 1.0)

            for t in list(range(ntiles)) * loops:
                idx_t = pool.tile([128, S, 128 * K // 16], i16)
                nc.sync.dma_start(idx_t[:], idxw_r[t])
                par_t = pool.tile([128, SK], f32)
                nc.sync.dma_start(par_t[:], parw_d[t])

                # Gather the pair-row for every (point, k): G2[p, (s k), :].
                G2 = gpool.tile([128, SK, PAIR], f32)
                for s in range(S) if not skip_gather else []:
                    nc.gpsimd.dma_gather(
                        out_ap=G2[:, s * K : (s + 1) * K, :],
                        in_ap=T2_d[:, :],
                        idxs_ap=idx_t[:, s, :],
                        num_idxs=128 * K,
                        num_idxs_reg=128 * K,
                        elem_size=PAIR,
                        single_packet=single_packet,
                    )

                if gather_only:
                    continue
                out_t = pool.tile([128, S, K, 2 * F], f32)
                X = pool.tile([128, S, XCP], f32)
                nc.sync.dma_start(X[:, :, 0:2], pcc_r[t])
                nc.vector.tensor_copy(
                    out=X[:, :, XC - 1 : XCP],
                    in_=onez[:].unsqueeze(1).to_broadcast([128, S, XCP - XC + 1]),
                )

                # Parity select (exact: par is 0.0 or 1.0):
                #   dst = lo + (hi - lo) * par
                # feats -> out_t[..., 0:F], neighbor pc -> X triples.
                sub = mybir.AluOpType.subtract
                mult = mybir.AluOpType.mult
                add = mybir.AluOpType.add
                of = out_t[:, :, :, 0:F]
                lo_f = G2[:, :, 0:F].rearrange("p (s k) c -> p s k c", k=K)
                hi_f = G2[:, :, TROW : TROW + F].rearrange(
                    "p (s k) c -> p s k c", k=K
                )
                par3 = par_t[:].rearrange("p (s k) -> p s k", k=K)
                par_f = par3.unsqueeze(3).to_broadcast([128, S, K, F])
                nc.vector.tensor_tensor(out=of, in0=hi_f, in1=lo_f, op=sub)
                nc.vector.tensor_tensor(out=of, in0=of, in1=par_f, op=mult)
                nc.vector.tensor_tensor(out=of, in0=of, in1=lo_f, op=add)

                trip = X[:, :, 2 : 2 + 3 * K].rearrange("p s (k c) -> p s k c", c=3)
                onp = trip[:, :, :, 0:2]
                lo_p = G2[:, :, F : F + 2].rearrange("p (s k) c -> p s k c", k=K)
                hi_p = G2[:, :, TROW + F : TROW + F + 2].rearrange(
                    "p (s k) c -> p s k c", k=K
                )
                par_p = par3.unsqueeze(3).to_broadcast([128, S, K, 2])
                nc.vector.tensor_tensor(out=onp, in0=hi_p, in1=lo_p, op=sub)
                nc.vector.tensor_tensor(out=onp, in0=onp, in1=par_p, op=mult)
                nc.vector.tensor_tensor(out=onp, in0=onp, in1=lo_p, op=add)

                npx = trip[:, :, :, 0:1].rearrange("p s k c -> p s (k c)")
                npy = trip[:, :, :, 1:2].rearrange("p s k c -> p s (k c)")
                nrm = trip[:, :, :, 2:3].rearrange("p s k c -> p s (k c)")
                cx = X[:, :, 0:1].to_broadcast([128, S, K])
                cy = X[:, :, 1:2].to_broadcast([128, S, K])

                dx = pool.tile([128, S, K], f32)
                dy = pool.tile([128, S, K], f32)
                nc.vector.tensor_tensor(out=dx[:], in0=cx, in1=npx, op=sub)
                nc.vector.tensor_tensor(out=dy[:], in0=cy, in1=npy, op=sub)
                nc.vector.tensor_tensor(out=dx[:], in0=dx[:], in1=dx[:], op=mult)
                nc.vector.tensor_tensor(out=dy[:], in0=dy[:], in1=dy[:], op=mult)
                nc.vector.tensor_tensor(out=dx[:], in0=dx[:], in1=dy[:], op=add)
                nc.scalar.activation(
                    out=nrm, in_=dx[:], func=mybir.ActivationFunctionType.Sqrt
                )

                # Transpose each sub-group X -> XT (contraction on partitions,
                # all at base partition 0 as matmul requires).
                xt_p = psum.tile([XCP, S, 128], f32)
                for s in range(S):
                    nc.tensor.transpose(
                        out=xt_p[:, s, :], in_=X[:, s, :], identity=ident[:]
                    )
                xt = pool.tile([XCP, S, 128], f32)
                nc.vector.tensor_copy(out=xt[:], in_=xt_p[:])

                r_p = psum.tile([128, S, K * F], f32)
                for s in range(S):
                    nc.tensor.matmul(
                        r_p[:, s, :],
                        lhsT=xt[:, s, :],
                        rhs=wf_sb[:],
                        start=True,
                        stop=True,
                    )

                nc.scalar.activation(
                    out=out_t[:, :, :, F : 2 * F],
                    in_=r_p[:].rearrange("p s (k f) -> p s k f", f=F),
                    func=mybir.ActivationFunctionType.Relu,
                )
                nc.sync.dma_start(
                    out=out_r[t], in_=out_t[:].rearrange("p s k f -> p (s k f)")
                )
            if timing_variant:
                dz = pool.tile([128, 1], f32)
                nc.vector.memset(dz[:], 0.0)
                nc.sync.dma_start(dummy_d[:, :], dz[:])

    nc.compile()
    return nc


def fold_weights(W, b):
    """Fold relp = Kpc - np into the weights; build the block-diag matrix."""
    W = np.asarray(W, np.float32)
    b = np.asarray(b, np.float32)
    Wc = W[0:2] + W[4:6]
    Wn = W[2:4] - W[4:6]
    Wr = W[6]
    Wf = np.zeros((XCP, K * F), np.float32)
    Wf[0] = np.tile(Wc[0], K)
    Wf[1] = np.tile(Wc[1], K)
    for k in range(K):
        Wf[2 + 3 * k, k * F : (k + 1) * F] = Wn[0]
        Wf[3 + 3 * k, k * F : (k + 1) * F] = Wn[1]
        Wf[4 + 3 * k, k * F : (k + 1) * F] = Wr
    Wf[XC - 1] = np.tile(b, K)
    return Wf


def pack_pair_table(feats_s, pc_s):
    """[feats[2m] | pc[2m] | feats[2m+1] | pc[2m+1] | pad] at 512B stride."""
    n = feats_s.shape[0]
    T2 = np.zeros((n // 2, PAIR), np.float32)
    T2[:, 0:F] = feats_s[0::2]
    T2[:, F : F + 2] = pc_s[0::2]
    T2[:, TROW : TROW + F] = feats_s[1::2]
    T2[:, TROW + F : TROW + F + 2] = pc_s[1::2]
    return T2


def marshal_indices(idx, ntiles):
    """idx (rows, K) -> wrapped int16 half-indices + float parity planes.

    One gather per (tile, sub-group): flat order g = k*128 + p; index g
    lives at partition g%16, free slot g//16, replicated across the eight
    16-partition groups.
    """
    idx = np.asarray(idx, np.int64)
    idx2 = (idx >> 1).astype(np.int16)
    par = (idx & 1).astype(np.float32)
    n1 = 128 * K  # indices per gather (one per sub-group)
    g = idx2.reshape(ntiles, S, 128, K).transpose(0, 1, 3, 2).reshape(ntiles, S, n1)
    idxw = np.ascontiguousarray(
        np.tile(
            g.reshape(ntiles, S, n1 // 16, 16).transpose(0, 1, 3, 2), (1, 1, 8, 1)
        ).reshape(ntiles * S * 128, n1 // 16)
    )
    parw = np.ascontiguousarray(
        par.reshape(ntiles, S, 128, K).transpose(0, 2, 1, 3).reshape(ntiles, 128, SK)
    )
    return idxw, parw


_PROGRAM = None


def _get_program():
    global _PROGRAM
    if _PROGRAM is None:
        _PROGRAM = build_program(ROWS_PER_CORE, N)
    return _PROGRAM


def make_in_maps(pc, feats, n_idx, W, b):
    pc = np.ascontiguousarray(np.asarray(pc, np.float32))
    feats = np.ascontiguousarray(np.asarray(feats, np.float32))
    n_idx = np.asarray(n_idx, np.int64)
    Wf = fold_weights(W, b)
    tables = [pack_pair_table(feats[s], pc[s]) for s in range(B)]
    ntiles = ROWS_PER_CORE // PTS_PER_TILE
    in_maps = []
    for c in range(NCORES):
        s, h = divmod(c, 2)
        sl = slice(h * ROWS_PER_CORE, (h + 1) * ROWS_PER_CORE)
        idxw, parw = marshal_indices(n_idx[s, sl], ntiles)
        in_maps.append(
            {
                "T2": tables[s],
                "idxw": idxw,
                "parw": parw,
                "pcc": np.ascontiguousarray(pc[s, sl]),
                "Wf": Wf,
            }
        )
    return in_maps


def kernel(pc, feats, n_idx, W, b):
    from concourse.bass_utils import run_bass_kernel_spmd

    nc = _get_program()
    in_maps = make_in_maps(pc, feats, n_idx, W, b)
    res = run_bass_kernel_spmd(nc, in_maps, list(range(NCORES)))
    out = np.empty((B, N, K, 2 * F), np.float32)
    for c in range(NCORES):
        s, h = divmod(c, 2)
        sl = slice(h * ROWS_PER_CORE, (h + 1) * ROWS_PER_CORE)
        out[s, sl] = res.results[c]["out"].reshape(ROWS_PER_CORE, K, 2 * F)
    return out

